# revision 6
# baseline (speedup 1.0000x reference)
"""DSA indexer kernel for Trainium2 (8 NeuronCores, SPMD over query dim).

Computes, for B=1, Sq=Sk=4096, H=16, D=128, topk=2048:
    logits = einsum("bqhd,bkd->bqhk", index_q, index_k) / sqrt(D)
    scores = einsum("bqhk,bqh->bqk", relu(logits), index_weights)
    scores = where(causal_mask, scores, -inf)
    topk_indices = top_k(scores, 2048).indices   (descending, ties -> lower idx)
returns (topk_indices int32 [1,4096,2048], scores f32 [1,4096,4096]).

Sharding: each of the 8 cores owns 512 consecutive queries; index_k is
replicated. Per core: per-head logits via fp32 PE matmuls (contraction d=128),
relu on ACT, head-sum via one-hot fp32 matmuls accumulating in PSUM, exact
-inf masking, then a per-row exact top-k sort on DVE:
u32 keys 0x40000000|(k18<<12)|(idx^0xFFF) (positive-normal f32 bit patterns,
so f32 max/min orders them exactly), 78-stage bitonic merge network, rank
inversion via per-partition local_scatter, 16-bit fractional refinement and
odd-even repair passes to restore full fp32 ordering among quantization ties.
"""
import math

import numpy as np

import concourse.bass as bass
from concourse import bacc
import concourse.mybir as mybir
from concourse.masks import make_identity
from concourse.tile import TileContext
from concourse import bass_utils

dt = mybir.dt
Alu = mybir.AluOpType

NP = 128
B, SQ, SK, H, D = 1, 4096, 4096, 16, 128
NCORES = 8
QPC = SQ // NCORES  # queries per core (512)
QT = QPC // NP  # q-tiles per core (4)
KC = 512  # k chunk width (PSUM bank)
NKC = SK // KC  # 8 chunks
TOPK = 2048
W = 2112  # repair window
N_REPAIR_PASSES = 12
KMAX = 2**18 - 2049

_cache = {}


def ceil_div(a, b):
    return (a + b - 1) // b


def _emit_topk(nc, pool, scores, P, K, Wn, idx_out_i32, tag_prefix=""):
    """Emit exact top-K index computation for scores[:, :P] (see module doc)."""
    tp = tag_prefix
    _n = [0]

    def tile(shape, dtype, tag):
        _n[0] += 1
        return pool.tile(shape, dtype, tag="srt" + tag, name=f"{tp}srt{_n[0]}_{tag}")

    f32 = dt.float32

    rmax = tile([NP, 1], f32, "rmax")
    nc.vector.tensor_reduce(rmax[:], scores[:, 0:P], axis=mybir.AxisListType.X, op=Alu.max)
    nc.vector.tensor_scalar(rmax[:], rmax[:], 1e-30, None, op0=Alu.max)
    scale = tile([NP, 1], f32, "scale")
    nc.vector.reciprocal(scale[:], rmax[:])
    nc.vector.tensor_scalar(scale[:], scale[:], float(KMAX - 1), None, op0=Alu.mult)

    t2 = tile([NP, P], f32, "A")
    nc.vector.tensor_scalar(t2[:], scores[:, 0:P], scale[:], 1.0, op0=Alu.mult, op1=Alu.add)
    nc.vector.tensor_scalar(t2[:], t2[:], 0.0, float(KMAX), op0=Alu.max, op1=Alu.min)
    k18 = tile([NP, P], dt.uint32, "B")
    nc.vector.tensor_copy(k18[:], t2[:])
    k18f = tile([NP, P], f32, "C")
    nc.vector.tensor_copy(k18f[:], k18[:])
    X = tile([NP, P], dt.uint32, "X")
    nc.vector.tensor_scalar(X[:], k18[:], 12, None, op0=Alu.logical_shift_left)
    fr32 = tile([NP, P], f32, "B")
    nc.vector.tensor_tensor(out=fr32[:], in0=t2[:], in1=k18f[:], op=Alu.subtract)
    fr16 = tile([NP, P], dt.uint16, "fr16")
    nc.vector.tensor_scalar(fr16[:], fr32[:], 0.5, float(2**15), op0=Alu.add, op1=Alu.mult)
    rev = tile([NP, P], dt.uint32, "A")
    nc.gpsimd.iota(rev[:], pattern=[[1, P]], base=0, channel_multiplier=0)
    nc.vector.tensor_scalar(rev[:], rev[:], 4095, 0x40000000, op0=Alu.bitwise_xor, op1=Alu.bitwise_or)
    nc.vector.tensor_tensor(out=X[:], in0=X[:], in1=rev[:], op=Alu.bitwise_or)

    Y = tile([NP, P], dt.uint32, "C")
    levels = int(math.log2(P))
    src, dst = X, Y

    def f(ap):
        return ap.bitcast(f32)

    for lev in range(1, levels + 1):
        L = 1 << (lev - 1)
        a = src[:].rearrange("p (a b) -> p a b", b=2 * L)
        o = dst[:].rearrange("p (a b) -> p a b", b=2 * L)
        nc.vector.tensor_tensor(
            out=f(o[:, :, 0:L]), in0=f(a[:, :, 0:L]), in1=f(a[:, :, 2 * L - 1 : L - 1 : -1]), op=Alu.max
        )
        nc.vector.tensor_tensor(
            out=f(o[:, :, L : 2 * L]), in0=f(a[:, :, L - 1 :: -1]), in1=f(a[:, :, L : 2 * L]), op=Alu.min
        )
        src, dst = dst, src
        d = L // 2
        while d >= 1:
            a = src[:].rearrange("p (a b) -> p a b", b=2 * d)
            o = dst[:].rearrange("p (a b) -> p a b", b=2 * d)
            nc.vector.tensor_tensor(
                out=f(o[:, :, 0:d]), in0=f(a[:, :, 0:d]), in1=f(a[:, :, d : 2 * d]), op=Alu.max
            )
            nc.vector.tensor_tensor(
                out=f(o[:, :, d : 2 * d]), in0=f(a[:, :, 0:d]), in1=f(a[:, :, d : 2 * d]), op=Alu.min
            )
            src, dst = dst, src
            d //= 2
    Ks = src

    idx16 = tile([NP, Wn], dt.uint16, "idx16")
    tmpu = tile([NP, Wn], dt.uint32, "tmpu")
    nc.vector.tensor_scalar(tmpu[:], Ks[:, 0:Wn], 4095, 4095, op0=Alu.bitwise_and, op1=Alu.bitwise_xor)
    nc.vector.tensor_copy(idx16[:], tmpu[:])

    ranks = tile([NP, Wn], dt.uint16, "ranks")
    nc.gpsimd.iota(ranks[:], pattern=[[1, Wn]], base=1, channel_multiplier=0)
    R = tile([NP, P], dt.uint16, "R")
    ci = tile([NP, Wn], dt.int16, "ci")
    cm = tile([NP, Wn], dt.int16, "cmask")
    for c in range(ceil_div(P, 1024)):
        nc.vector.tensor_scalar(ci[:], idx16[:], 1024 * c, None, op0=Alu.subtract)
        nc.vector.tensor_scalar(cm[:], ci[:], 1024, None, op0=Alu.is_ge)
        nc.vector.tensor_scalar(cm[:], cm[:], 15, None, op0=Alu.logical_shift_left)
        nc.vector.tensor_tensor(out=ci[:], in0=ci[:], in1=cm[:], op=Alu.bitwise_or)
        nc.gpsimd.local_scatter(
            R[:, c * 1024 : (c + 1) * 1024], ranks[:], ci[:],
            channels=NP, num_elems=1024, num_idxs=Wn,
        )

    frs = tile([NP, Wn], dt.uint16, "frs")
    half = Wn // 2
    rm1 = tile([NP, P], dt.int16, "rm1")
    rmm = tile([NP, P], dt.int16, "cmask")
    for c in range(2):
        nc.vector.tensor_scalar(rm1[:], R[:], 1 + half * c, None, op0=Alu.subtract)
        nc.vector.tensor_scalar(rmm[:], rm1[:], half, None, op0=Alu.is_ge)
        nc.vector.tensor_scalar(rmm[:], rmm[:], 15, None, op0=Alu.logical_shift_left)
        nc.vector.tensor_tensor(out=rm1[:], in0=rm1[:], in1=rmm[:], op=Alu.bitwise_or)
        nc.gpsimd.local_scatter(
            frs[:, c * half : (c + 1) * half], fr16[:], rm1[:],
            channels=NP, num_elems=half, num_idxs=P,
        )

    Cref = tile([NP, Wn], dt.uint32, "Cref")
    nc.vector.tensor_copy(tmpu[:], frs[:])  # u16 -> u32 cast
    nc.vector.tensor_scalar(tmpu[:], tmpu[:], 4, None, op0=Alu.logical_shift_right)
    nc.vector.tensor_scalar(Cref[:], Ks[:, 0:Wn], 0xFFFFF000, None, op0=Alu.bitwise_and)
    nc.vector.tensor_tensor(out=Cref[:], in0=Cref[:], in1=tmpu[:], op=Alu.bitwise_or)

    mpred = tile([NP, Wn], dt.uint16, "mpred")
    tsw = tile([NP, Wn], dt.uint16, "tsw")
    for p_i in range(N_REPAIR_PASSES):
        off = p_i % 2
        n_el = Wn - off
        n_el -= n_el % 2
        npair = n_el // 2
        cA = Cref[:, off : off + n_el].rearrange("p (a b) -> p a b", b=2)
        iA = idx16[:, off : off + n_el].rearrange("p (a b) -> p a b", b=2)
        mp = mpred[:, :npair]
        nc.vector.tensor_tensor(out=mp, in0=f(cA[:, :, 1:2]), in1=f(cA[:, :, 0:1]), op=Alu.is_gt)
        t32 = tmpu[:, :npair]
        nc.vector.tensor_tensor(out=f(t32), in0=f(cA[:, :, 0:1]), in1=f(cA[:, :, 1:2]), op=Alu.max)
        nc.vector.tensor_tensor(out=f(cA[:, :, 1:2]), in0=f(cA[:, :, 0:1]), in1=f(cA[:, :, 1:2]), op=Alu.min)
        nc.vector.tensor_copy(cA[:, :, 0:1], t32)
        t16 = tsw[:, :npair]
        nc.vector.tensor_copy(t16, iA[:, :, 0:1])
        nc.vector.copy_predicated(iA[:, :, 0:1], mp, iA[:, :, 1:2])
        nc.vector.copy_predicated(iA[:, :, 1:2], mp, t16)

    nc.vector.tensor_copy(idx_out_i32[:], idx16[:, 0:K])


def build_nc(n_qtiles=QT):
    nc = bacc.Bacc("TRN2", target_bir_lowering=False, debug=False, num_devices=NCORES)
    f32 = dt.float32
    q_d = nc.dram_tensor("q", [n_qtiles * NP * H, D], f32, kind="ExternalInput")
    k_d = nc.dram_tensor("k", [SK, D], f32, kind="ExternalInput")
    w_d = nc.dram_tensor("w", [n_qtiles * NP * H, 1], f32, kind="ExternalInput")
    m_d = nc.dram_tensor("m", [n_qtiles * NP, SK], dt.uint8, kind="ExternalInput")
    sc_d = nc.dram_tensor("scores", [n_qtiles * NP, SK], f32, kind="ExternalOutput")
    idx_d = nc.dram_tensor("idx", [n_qtiles * NP, TOPK], dt.int32, kind="ExternalOutput")

    wscale = float(D) ** -0.5

    with TileContext(nc) as tc:
        with (
            tc.tile_pool(name="const", bufs=1) as cpool,
            tc.tile_pool(name="work", bufs=1) as pool,
            tc.tile_pool(name="ld", bufs=3) as ldpool,
            tc.tile_pool(name="relu", bufs=3) as rpool,
            tc.tile_pool(name="psq", bufs=2, space="PSUM") as psq,
            tc.tile_pool(name="pslg", bufs=2, space="PSUM") as pslg,
            tc.tile_pool(name="pssc", bufs=2, space="PSUM") as pssc,
        ):
            ident = cpool.tile([NP, NP], f32)
            make_identity(nc, ident[:])

            # one-hot head-sum pattern: wide[p, c] = 1 iff c == 128 + (p>>4)
            wide = cpool.tile([NP, 2 * NP], f32)
            col = pool.tile([NP, 2 * NP], dt.uint32, tag="srtA", name="oh_col")
            pio = cpool.tile([NP, 1], dt.uint32)
            nc.gpsimd.iota(col[:], pattern=[[1, 2 * NP]], base=0, channel_multiplier=0)
            nc.gpsimd.iota(pio[:], pattern=[[0, 1]], base=0, channel_multiplier=1)
            nc.vector.tensor_scalar(pio[:], pio[:], 4, None, op0=Alu.logical_shift_right)
            nc.vector.tensor_scalar(pio[:], pio[:], 128, None, op0=Alu.add)
            nc.vector.tensor_tensor(out=wide[:], in0=col[:], in1=pio[:].to_broadcast([NP, 2 * NP]), op=Alu.is_equal)

            # kT: transpose k -> [128 d, SK]
            kTr = cpool.tile([NP, SK], f32)
            for j in range(SK // NP):
                kl = ldpool.tile([NP, D], f32, tag="kload", name=f"kl{j}")
                nc.sync.dma_start(kl[:], k_d[j * NP : (j + 1) * NP, :])
                kt_ps = psq.tile([NP, NP], f32, tag="ktp", name=f"ktp{j}")
                nc.tensor.transpose(kt_ps[:], kl[:], ident[:])
                nc.scalar.copy(kTr[:, j * NP : (j + 1) * NP], kt_ps[:])

            for t in range(n_qtiles):
                # strided sharding: tile t of every core covers global queries
                # m + 8*(128t + i)  ->  max query index 1024(t+1)-1.
                nkc_t = min(NKC, 2 * (t + 1)) if n_qtiles == QT else NKC
                p_t = 2048 if (n_qtiles == QT and t < 2) else SK
                # load + scale + transpose q for this tile -> qTall [128 d, 2048 qh]
                qTall = pool.tile([NP, H * NP], f32, tag="qTall", name=f"qT{t}")
                for u in range(H):
                    row0 = (t * H + u) * NP
                    ql = ldpool.tile([NP, D], f32, tag="qload", name=f"ql{t}_{u}")
                    nc.sync.dma_start(ql[:], q_d[row0 : row0 + NP, :])
                    wl = ldpool.tile([NP, 1], f32, tag="wload", name=f"wl{t}_{u}")
                    nc.sync.dma_start(wl[:], w_d[row0 : row0 + NP, :])
                    nc.vector.tensor_scalar(wl[:], wl[:], wscale, None, op0=Alu.mult)
                    nc.vector.tensor_scalar(ql[:], ql[:], wl[:], None, op0=Alu.mult)
                    qt_ps = psq.tile([NP, NP], f32, tag="qtp", name=f"qtp{t}_{u}")
                    nc.tensor.transpose(qt_ps[:], ql[:], ident[:])
                    nc.scalar.copy(qTall[:, u * NP : (u + 1) * NP], qt_ps[:])

                sc_sb = pool.tile([NP, SK], f32, tag="scsb", name=f"sc{t}")
                if nkc_t < NKC:
                    nc.vector.memset(sc_sb[:, nkc_t * KC :], float("-inf"))
                for c in range(nkc_t):
                    mskc = ldpool.tile([NP, KC], dt.uint8, tag="mskc", name=f"msk{t}_{c}")
                    nc.sync.dma_start(mskc[:], m_d[t * NP : (t + 1) * NP, c * KC : (c + 1) * KC])
                    caddc = ldpool.tile([NP, KC], f32, tag="caddc", name=f"cadd{t}_{c}")
                    nc.vector.tensor_scalar(caddc[:], mskc[:], 1.0, 3.4e38, op0=Alu.subtract, op1=Alu.mult)
                    sc_ps = pssc.tile([NP, KC], f32, tag="scps", name=f"scps{t}_{c}")
                    for u in range(H):
                        lg_ps = pslg.tile([NP, KC], f32, tag="lgps", name=f"lg{t}_{c}_{u}")
                        nc.tensor.matmul(
                            lg_ps[:], qTall[:, u * NP : (u + 1) * NP],
                            kTr[:, c * KC : (c + 1) * KC], start=True, stop=True,
                        )
                        rl = rpool.tile([NP, KC], f32, tag="rl", name=f"rl{t}_{c}_{u}")
                        nc.scalar.activation(rl[:], lg_ps[:], mybir.ActivationFunctionType.Relu)
                        nc.tensor.matmul(
                            sc_ps[:], wide[:, NP - 8 * u : 2 * NP - 8 * u], rl[:],
                            start=(u == 0), stop=(u == H - 1),
                        )
                    nc.vector.scalar_tensor_tensor(
                        sc_sb[:, c * KC : (c + 1) * KC], caddc[:],
                        2.0, sc_ps[:], op0=Alu.mult, op1=Alu.add,
                    )
                nc.sync.dma_start(sc_d[t * NP : (t + 1) * NP, :], sc_sb[:])

                idx_sb = pool.tile([NP, TOPK], dt.int32, tag="idxsb", name=f"idx{t}")
                w_t = min(W, p_t)
                _emit_topk(nc, pool, sc_sb[:], p_t, TOPK, w_t, idx_sb[:], tag_prefix=f"t{t}")
                nc.sync.dma_start(idx_d[t * NP : (t + 1) * NP, :], idx_sb[:])

    nc.finalize()
    return nc


def kernel(index_q, index_k, index_weights, attn_mask, topk):
    assert int(topk) == TOPK
    index_q = np.ascontiguousarray(np.asarray(index_q, dtype=np.float32))
    index_k = np.ascontiguousarray(np.asarray(index_k, dtype=np.float32))
    index_weights = np.ascontiguousarray(np.asarray(index_weights, dtype=np.float32))
    mask_u8 = np.ascontiguousarray(np.asarray(attn_mask)).view(np.uint8)

    if "nc" not in _cache:
        _cache["nc"] = build_nc()
    nc = _cache["nc"]

    k_full = index_k[0]  # [SK, D]
    in_maps = []
    for c in range(NCORES):
        qs = np.ascontiguousarray(index_q[0, c::NCORES]).reshape(QPC * H, D)
        ws = np.ascontiguousarray(index_weights[0, c::NCORES]).reshape(QPC * H, 1)
        ms = np.ascontiguousarray(mask_u8[c::NCORES])
        in_maps.append({"q": qs, "k": k_full, "w": ws, "m": ms})

    res = bass_utils.run_bass_kernel_spmd(nc, in_maps, core_ids=list(range(NCORES)))
    _cache["last_results"] = res
    idx = np.empty((SQ, TOPK), np.int32)
    scores = np.empty((SQ, SK), np.float32)
    for c in range(NCORES):
        idx[c::NCORES] = res.results[c]["idx"]
        scores[c::NCORES] = res.results[c]["scores"]
    return idx[None], scores[None]


# revision 8
# speedup vs baseline: 1.1256x; 1.1256x over previous
"""DSA indexer kernel for Trainium2 (8 NeuronCores, SPMD over query dim).

Computes, for B=1, Sq=Sk=4096, H=16, D=128, topk=2048:
    logits = einsum("bqhd,bkd->bqhk", index_q, index_k) / sqrt(D)
    scores = einsum("bqhk,bqh->bqk", relu(logits), index_weights)
    scores = where(causal_mask, scores, -inf)
    topk_indices = top_k(scores, 2048).indices   (descending, ties -> lower idx)
returns (topk_indices int32 [1,4096,2048], scores f32 [1,4096,4096]).

Sharding: each of the 8 cores owns 512 consecutive queries; index_k is
replicated. Per core: per-head logits via fp32 PE matmuls (contraction d=128),
relu on ACT, head-sum via one-hot fp32 matmuls accumulating in PSUM, exact
-inf masking, then a per-row exact top-k sort on DVE:
u32 keys 0x40000000|(k18<<12)|(idx^0xFFF) (positive-normal f32 bit patterns,
so f32 max/min orders them exactly), 78-stage bitonic merge network, rank
inversion via per-partition local_scatter, 16-bit fractional refinement and
odd-even repair passes to restore full fp32 ordering among quantization ties.
"""
import math

import numpy as np

import concourse.bass as bass
from concourse import bacc
import concourse.mybir as mybir
from concourse.masks import make_identity
from concourse.tile import TileContext
from concourse import bass_utils

dt = mybir.dt
Alu = mybir.AluOpType
AF = mybir.ActivationFunctionType

NP = 128
B, SQ, SK, H, D = 1, 4096, 4096, 16, 128
NCORES = 8
QPC = SQ // NCORES  # queries per core (512)
QT = QPC // NP  # q-tiles per core (4)
KC = 512  # k chunk width (PSUM bank)
NKC = SK // KC  # 8 chunks
TOPK = 2048
W = 2112  # repair window
N_REPAIR_PASSES = 8
KMAX = 2**18 - 2049

_cache = {}


def ceil_div(a, b):
    return (a + b - 1) // b


def _emit_topk(nc, pool, scores, P, K, Wn, idx_out_i32, tag_prefix=""):
    """Emit exact top-K index computation for scores[:, :P] (see module doc)."""
    tp = tag_prefix
    _n = [0]

    def tile(shape, dtype, tag):
        _n[0] += 1
        return pool.tile(shape, dtype, tag="srt" + tag, name=f"{tp}srt{_n[0]}_{tag}")

    f32 = dt.float32

    rmax = tile([NP, 1], f32, "rmax")
    nc.vector.tensor_reduce(rmax[:], scores[:, 0:P], axis=mybir.AxisListType.X, op=Alu.max)
    nc.vector.tensor_scalar(rmax[:], rmax[:], 1e-30, None, op0=Alu.max)
    scale = tile([NP, 1], f32, "scale")
    nc.vector.reciprocal(scale[:], rmax[:])
    nc.vector.tensor_scalar(scale[:], scale[:], float(KMAX - 1), None, op0=Alu.mult)

    t2 = tile([NP, P], f32, "A")
    nc.scalar.activation(t2[:], scores[:, 0:P], AF.Copy, bias=1.0, scale=scale[:])
    nc.vector.tensor_scalar(t2[:], t2[:], 0.0, float(KMAX), op0=Alu.max, op1=Alu.min)
    k18 = tile([NP, P], dt.uint32, "B")
    nc.vector.tensor_copy(k18[:], t2[:])
    k18f = tile([NP, P], f32, "C")
    nc.scalar.copy(k18f[:], k18[:])
    X = tile([NP, P], dt.uint32, "X")
    nc.vector.tensor_scalar(X[:], k18[:], 12, None, op0=Alu.logical_shift_left)
    fr32 = tile([NP, P], f32, "B")
    nc.vector.tensor_tensor(out=fr32[:], in0=t2[:], in1=k18f[:], op=Alu.subtract)
    fr16 = tile([NP, P], dt.uint16, "fr16")
    nc.scalar.activation(fr16[:], fr32[:], AF.Copy, bias=16384.0, scale=32768.0)
    rev = tile([NP, P], dt.uint32, "A")
    nc.gpsimd.iota(rev[:], pattern=[[1, P]], base=0, channel_multiplier=0)
    nc.vector.tensor_scalar(rev[:], rev[:], 4095, 0x40000000, op0=Alu.bitwise_xor, op1=Alu.bitwise_or)
    nc.vector.tensor_tensor(out=X[:], in0=X[:], in1=rev[:], op=Alu.bitwise_or)

    Y = tile([NP, P], dt.uint32, "C")
    levels = int(math.log2(P))
    src, dst = X, Y

    def f(ap):
        return ap.bitcast(f32)

    for lev in range(1, levels + 1):
        L = 1 << (lev - 1)
        a = src[:].rearrange("p (a b) -> p a b", b=2 * L)
        o = dst[:].rearrange("p (a b) -> p a b", b=2 * L)
        nc.vector.tensor_tensor(
            out=f(o[:, :, 0:L]), in0=f(a[:, :, 0:L]), in1=f(a[:, :, 2 * L - 1 : L - 1 : -1]), op=Alu.max
        )
        nc.vector.tensor_tensor(
            out=f(o[:, :, L : 2 * L]), in0=f(a[:, :, L - 1 :: -1]), in1=f(a[:, :, L : 2 * L]), op=Alu.min
        )
        src, dst = dst, src
        d = L // 2
        while d >= 1:
            a = src[:].rearrange("p (a b) -> p a b", b=2 * d)
            o = dst[:].rearrange("p (a b) -> p a b", b=2 * d)
            nc.vector.tensor_tensor(
                out=f(o[:, :, 0:d]), in0=f(a[:, :, 0:d]), in1=f(a[:, :, d : 2 * d]), op=Alu.max
            )
            nc.vector.tensor_tensor(
                out=f(o[:, :, d : 2 * d]), in0=f(a[:, :, 0:d]), in1=f(a[:, :, d : 2 * d]), op=Alu.min
            )
            src, dst = dst, src
            d //= 2
    Ks = src

    idx16 = tile([NP, Wn], dt.uint16, "idx16")
    tmpu = tile([NP, Wn], dt.uint32, "tmpu")
    nc.vector.tensor_scalar(tmpu[:], Ks[:, 0:Wn], 4095, 4095, op0=Alu.bitwise_and, op1=Alu.bitwise_xor)
    nc.scalar.copy(idx16[:], tmpu[:])

    ranks = tile([NP, Wn], dt.uint16, "ranks")
    nc.gpsimd.iota(ranks[:], pattern=[[1, Wn]], base=1, channel_multiplier=0)
    R = tile([NP, P], dt.uint16, "R")
    ci = tile([NP, Wn], dt.int16, "ci")
    cm = tile([NP, Wn], dt.int16, "cmask")
    for c in range(ceil_div(P, 1024)):
        nc.vector.tensor_scalar(ci[:], idx16[:], 1024 * c, None, op0=Alu.subtract)
        nc.vector.tensor_scalar(cm[:], ci[:], 1024, None, op0=Alu.is_ge)
        nc.vector.tensor_scalar(cm[:], cm[:], 15, None, op0=Alu.logical_shift_left)
        nc.vector.tensor_tensor(out=ci[:], in0=ci[:], in1=cm[:], op=Alu.bitwise_or)
        nc.gpsimd.local_scatter(
            R[:, c * 1024 : (c + 1) * 1024], ranks[:], ci[:],
            channels=NP, num_elems=1024, num_idxs=Wn,
        )

    frs = tile([NP, Wn], dt.uint16, "frs")
    half = Wn // 2
    rm1 = tile([NP, P], dt.int16, "rm1")
    rmm = tile([NP, P], dt.int16, "cmask")
    for c in range(2):
        nc.vector.tensor_scalar(rm1[:], R[:], 1 + half * c, None, op0=Alu.subtract)
        nc.vector.tensor_scalar(rmm[:], rm1[:], half, None, op0=Alu.is_ge)
        nc.vector.tensor_scalar(rmm[:], rmm[:], 15, None, op0=Alu.logical_shift_left)
        nc.vector.tensor_tensor(out=rm1[:], in0=rm1[:], in1=rmm[:], op=Alu.bitwise_or)
        nc.gpsimd.local_scatter(
            frs[:, c * half : (c + 1) * half], fr16[:], rm1[:],
            channels=NP, num_elems=half, num_idxs=P,
        )

    Cref = tile([NP, Wn], dt.uint32, "Cref")
    nc.scalar.copy(tmpu[:], frs[:])  # u16 -> u32 cast (ACT)
    nc.vector.tensor_scalar(tmpu[:], tmpu[:], 4, None, op0=Alu.logical_shift_right)
    nc.vector.tensor_scalar(Cref[:], Ks[:, 0:Wn], 0xFFFFF000, None, op0=Alu.bitwise_and)
    nc.vector.tensor_tensor(out=Cref[:], in0=Cref[:], in1=tmpu[:], op=Alu.bitwise_or)

    mpred = tile([NP, Wn], dt.uint16, "mpred")
    tsw = tile([NP, Wn], dt.uint16, "tsw")
    C2 = tile([NP, Wn], dt.uint32, "B")
    csrc, cdst = Cref, C2
    for p_i in range(N_REPAIR_PASSES):
        off = p_i % 2
        n_el = Wn - off
        n_el -= n_el % 2
        npair = n_el // 2
        cA = csrc[:, off : off + n_el].rearrange("p (a b) -> p a b", b=2)
        cO = cdst[:, off : off + n_el].rearrange("p (a b) -> p a b", b=2)
        iA = idx16[:, off : off + n_el].rearrange("p (a b) -> p a b", b=2)
        mp = mpred[:, :npair]
        nc.vector.tensor_tensor(out=mp, in0=f(cA[:, :, 1:2]), in1=f(cA[:, :, 0:1]), op=Alu.is_gt)
        nc.vector.tensor_tensor(out=f(cO[:, :, 0:1]), in0=f(cA[:, :, 0:1]), in1=f(cA[:, :, 1:2]), op=Alu.max)
        nc.vector.tensor_tensor(out=f(cO[:, :, 1:2]), in0=f(cA[:, :, 0:1]), in1=f(cA[:, :, 1:2]), op=Alu.min)
        if off == 1:
            nc.scalar.copy(cdst[:, 0:1], csrc[:, 0:1])
            nc.scalar.copy(cdst[:, Wn - 1 : Wn], csrc[:, Wn - 1 : Wn])
        t16 = tsw[:, :npair]
        nc.scalar.copy(t16, iA[:, :, 0:1])
        nc.vector.copy_predicated(iA[:, :, 0:1], mp, iA[:, :, 1:2])
        nc.vector.copy_predicated(iA[:, :, 1:2], mp, t16)
        csrc, cdst = cdst, csrc

    nc.scalar.copy(idx_out_i32[:], idx16[:, 0:K])


def build_nc(n_qtiles=QT, causal=True):
    nc = bacc.Bacc("TRN2", target_bir_lowering=False, debug=False, num_devices=NCORES)
    f32 = dt.float32
    q_d = nc.dram_tensor("q", [n_qtiles * NP * H, D], f32, kind="ExternalInput")
    k_d = nc.dram_tensor("k", [SK, D], f32, kind="ExternalInput")
    w_d = nc.dram_tensor("w", [n_qtiles * NP * H, 1], f32, kind="ExternalInput")
    m_d = nc.dram_tensor("m", [n_qtiles * NP, SK], dt.uint8, kind="ExternalInput")
    sc_d = nc.dram_tensor("scores", [n_qtiles * NP, SK], f32, kind="ExternalOutput")
    idx_d = nc.dram_tensor("idx", [n_qtiles * NP, TOPK], dt.int32, kind="ExternalOutput")

    wscale = float(D) ** -0.5

    with TileContext(nc) as tc:
        with (
            tc.tile_pool(name="const", bufs=1) as cpool,
            tc.tile_pool(name="work", bufs=1) as pool,
            tc.tile_pool(name="ld", bufs=3) as ldpool,
            tc.tile_pool(name="relu", bufs=3) as rpool,
            tc.tile_pool(name="psq", bufs=2, space="PSUM") as psq,
            tc.tile_pool(name="pslg", bufs=2, space="PSUM") as pslg,
            tc.tile_pool(name="pssc", bufs=2, space="PSUM") as pssc,
        ):
            ident = cpool.tile([NP, NP], f32)
            make_identity(nc, ident[:])

            # one-hot head-sum pattern: wide[p, c] = 1 iff c == 128 + (p>>4)
            wide = cpool.tile([NP, 2 * NP], f32)
            col = pool.tile([NP, 2 * NP], dt.uint32, tag="srtA", name="oh_col")
            pio = cpool.tile([NP, 1], dt.uint32)
            nc.gpsimd.iota(col[:], pattern=[[1, 2 * NP]], base=0, channel_multiplier=0)
            nc.gpsimd.iota(pio[:], pattern=[[0, 1]], base=0, channel_multiplier=1)
            nc.vector.tensor_scalar(pio[:], pio[:], 4, None, op0=Alu.logical_shift_right)
            nc.vector.tensor_scalar(pio[:], pio[:], 128, None, op0=Alu.add)
            nc.vector.tensor_tensor(out=wide[:], in0=col[:], in1=pio[:].to_broadcast([NP, 2 * NP]), op=Alu.is_equal)

            # kT: transpose k -> [128 d, SK]
            kTr = cpool.tile([NP, SK], f32)
            for j in range(SK // NP):
                kl = ldpool.tile([NP, D], f32, tag="kload", name=f"kl{j}")
                nc.sync.dma_start(kl[:], k_d[j * NP : (j + 1) * NP, :])
                kt_ps = psq.tile([NP, NP], f32, tag="ktp", name=f"ktp{j}")
                nc.tensor.transpose(kt_ps[:], kl[:], ident[:])
                nc.scalar.copy(kTr[:, j * NP : (j + 1) * NP], kt_ps[:])

            for t in range(n_qtiles):
                # strided sharding: tile t of every core covers global queries
                # m + 8*(128t + i)  ->  max query index 1024(t+1)-1.
                nkc_t = min(NKC, 2 * (t + 1)) if (causal and n_qtiles == QT) else NKC
                p_t = 2048 if (causal and n_qtiles == QT and t < 2) else SK
                # load + scale + transpose q for this tile -> qTall [128 d, 2048 qh]
                qTall = pool.tile([NP, H * NP], f32, tag="qTall", name=f"qT{t}")
                for u in range(H):
                    row0 = (t * H + u) * NP
                    ql = ldpool.tile([NP, D], f32, tag="qload", name=f"ql{t}_{u}")
                    nc.sync.dma_start(ql[:], q_d[row0 : row0 + NP, :])
                    wl = ldpool.tile([NP, 1], f32, tag="wload", name=f"wl{t}_{u}")
                    nc.sync.dma_start(wl[:], w_d[row0 : row0 + NP, :])
                    nc.vector.tensor_scalar(wl[:], wl[:], wscale, None, op0=Alu.mult)
                    nc.vector.tensor_scalar(ql[:], ql[:], wl[:], None, op0=Alu.mult)
                    qt_ps = psq.tile([NP, NP], f32, tag="qtp", name=f"qtp{t}_{u}")
                    nc.tensor.transpose(qt_ps[:], ql[:], ident[:])
                    nc.scalar.copy(qTall[:, u * NP : (u + 1) * NP], qt_ps[:])

                sc_sb = pool.tile([NP, SK], f32, tag="scsb", name=f"sc{t}")
                if nkc_t < NKC:
                    nc.gpsimd.memset(sc_sb[:, nkc_t * KC :], float("-inf"))
                for c in range(nkc_t):
                    mskc = ldpool.tile([NP, KC], dt.uint8, tag="mskc", name=f"msk{t}_{c}")
                    nc.sync.dma_start(mskc[:], m_d[t * NP : (t + 1) * NP, c * KC : (c + 1) * KC])
                    caddc = ldpool.tile([NP, KC], f32, tag="caddc", name=f"cadd{t}_{c}")
                    nc.scalar.activation(caddc[:], mskc[:], AF.Copy, bias=-3.4e38, scale=3.4e38)
                    sc_ps = pssc.tile([NP, KC], f32, tag="scps", name=f"scps{t}_{c}")
                    for u in range(H):
                        lg_ps = pslg.tile([NP, KC], f32, tag="lgps", name=f"lg{t}_{c}_{u}")
                        nc.tensor.matmul(
                            lg_ps[:], qTall[:, u * NP : (u + 1) * NP],
                            kTr[:, c * KC : (c + 1) * KC], start=True, stop=True,
                        )
                        rl = rpool.tile([NP, KC], f32, tag="rl", name=f"rl{t}_{c}_{u}")
                        nc.scalar.activation(rl[:], lg_ps[:], mybir.ActivationFunctionType.Relu)
                        nc.tensor.matmul(
                            sc_ps[:], wide[:, NP - 8 * u : 2 * NP - 8 * u], rl[:],
                            start=(u == 0), stop=(u == H - 1),
                        )
                    nc.vector.scalar_tensor_tensor(
                        sc_sb[:, c * KC : (c + 1) * KC], caddc[:],
                        2.0, sc_ps[:], op0=Alu.mult, op1=Alu.add,
                    )
                nc.sync.dma_start(sc_d[t * NP : (t + 1) * NP, :], sc_sb[:])

                idx_sb = pool.tile([NP, TOPK], dt.int32, tag="idxsb", name=f"idx{t}")
                w_t = min(W, p_t)
                _emit_topk(nc, pool, sc_sb[:], p_t, TOPK, w_t, idx_sb[:], tag_prefix=f"t{t}")
                nc.sync.dma_start(idx_d[t * NP : (t + 1) * NP, :], idx_sb[:])

    nc.finalize()
    return nc


def kernel(index_q, index_k, index_weights, attn_mask, topk):
    assert int(topk) == TOPK
    index_q = np.ascontiguousarray(np.asarray(index_q, dtype=np.float32))
    index_k = np.ascontiguousarray(np.asarray(index_k, dtype=np.float32))
    index_weights = np.ascontiguousarray(np.asarray(index_weights, dtype=np.float32))
    mask_u8 = np.ascontiguousarray(np.asarray(attn_mask)).view(np.uint8)

    # The fast path skips fully-masked key chunks and sorts a 2048-wide window
    # for early query tiles; both are only valid for the causal (tril) mask.
    causal = bool((mask_u8 == np.tril(np.ones((SQ, SK), np.uint8))).all())
    key = "nc_causal" if causal else "nc_full"
    if key not in _cache:
        _cache[key] = build_nc(causal=causal)
    nc = _cache[key]

    k_full = index_k[0]  # [SK, D]
    in_maps = []
    for c in range(NCORES):
        qs = np.ascontiguousarray(index_q[0, c::NCORES]).reshape(QPC * H, D)
        ws = np.ascontiguousarray(index_weights[0, c::NCORES]).reshape(QPC * H, 1)
        ms = np.ascontiguousarray(mask_u8[c::NCORES])
        in_maps.append({"q": qs, "k": k_full, "w": ws, "m": ms})

    res = bass_utils.run_bass_kernel_spmd(nc, in_maps, core_ids=list(range(NCORES)))
    _cache["last_results"] = res
    idx = np.empty((SQ, TOPK), np.int32)
    scores = np.empty((SQ, SK), np.float32)
    for c in range(NCORES):
        idx[c::NCORES] = res.results[c]["idx"]
        scores[c::NCORES] = res.results[c]["scores"]
    return idx[None], scores[None]
